# revision 8
# baseline (speedup 1.0000x reference)
"""ResGatedGraphConv (gnn_message_passing) Trainium2 Bass kernel, 8-core SPMD.

Strategy:
  - Host: sort edges by dst; shard 128-node windows contiguously over 8 cores
    balancing edge counts (each core owns a node range -> no all-reduce).
  - Device per core:
      pass0: build qv table [N,128] = [q/2 | v] and k/2 table for local range
             (PE matmuls from host-pretransposed x).
      loop: per 128-edge tile: gather k/2[dst_local], qv[src] (indirect DMA),
            e = eaT_tile.T @ [We.T;be] (PE, psum), s=(kd+qs) t=s+e vv=vs+e
            (DVE), g=sigmoid(2t) (ACT), msg=g*vv (DVE),
            sel one-hot = (iota == dst_in_window) (DVE/GPSIMD),
            window psum += sel.T @ msg (PE accumulate).
        per window (T_w tiles): psum += xT_win.T @ [Ws.T;bs] (skip),
            out = relu(psum) (ACT), DMA out rows.
  - Host: concat per-core row ranges.
"""

import os
import sys
import math

import numpy as np

for _p in ("/opt/trn_rl_repo",):
    if _p not in sys.path:
        sys.path.append(_p)

import ml_dtypes  # noqa: E402

BF16 = ml_dtypes.bfloat16

# problem constants (hardcoded per spec)
N_NODES = 100000
N_EDGES = 1000000
D = 64
NCORES = 8
WIN = 128  # nodes per aggregation window (= one-hot selector width)

# tunables (env-overridable for experiments)
GROUP_TILES = int(os.environ.get("GNN_GROUP_TILES", "64"))  # tiles per gather group
SEL_MODE = os.environ.get("GNN_SEL", "vector")  # vector | gpsimd | alt
TABLE_DT = os.environ.get("GNN_TABLE_DT", "bf16")  # bf16 | f32  (tables+compute)


def _cdiv(a, b):
    return (a + b - 1) // b


# ----------------------------------------------------------------------------
# host-side preprocessing
# ----------------------------------------------------------------------------

def build_host_data(x, edge_index, edge_attr, Wk, bk, Wq, bq, Wv, bv, We, be,
                    Ws, bs, n_nodes, ncores):
    """Sort/shard edges, build per-core input arrays + structural meta."""
    n = n_nodes
    src = np.asarray(edge_index[0], dtype=np.int64)
    dst = np.asarray(edge_index[1], dtype=np.int64)
    ne = src.shape[0]

    order = np.argsort(dst, kind="stable")
    src_s = src[order].astype(np.int32)
    dst_s = dst[order].astype(np.int32)

    w_total = _cdiv(n, WIN)
    win_of_edge = dst_s >> 7
    counts = np.bincount(win_of_edge, minlength=w_total).astype(np.int64)
    cum = np.concatenate([[0], np.cumsum(counts)])

    # contiguous window runs with ~equal edge counts
    targets = (np.arange(1, ncores) * ne) // ncores
    cuts = np.searchsorted(cum, targets)  # window index cuts
    wstart = np.concatenate([[0], cuts, [w_total]]).astype(np.int64)
    w_per_core = np.diff(wstart)
    w_max = int(w_per_core.max())
    t_w = int(max(1, _cdiv(int(counts.max()), 128)))  # tiles per window (global max)
    t_pad = w_max * t_w

    np_tab = BF16 if TABLE_DT == "bf16" else np.float32

    # global padded xT (for qv table build); [65, n_pad0]
    n_pad0 = w_total * WIN
    xT_aug = np.zeros((D + 1, n_pad0), dtype=np_tab)
    xT_aug[:D, :n] = x.T.astype(np_tab)
    xT_aug[D, :n] = 1.0

    # weight packs
    def pack(w, b, scale):
        p = np.zeros((D + 1, D), dtype=np_tab)
        p[:D] = (w.T * scale).astype(np_tab)
        p[D] = (b * scale).astype(np_tab)
        return p

    w_e_pack = pack(We, be, 1.0)
    w_k_pack = pack(Wk, bk, 0.5)
    w_s_pack = pack(Ws, bs, 1.0)
    w_qv_pack = np.concatenate([pack(Wq, bq, 0.5), pack(Wv, bv, 1.0)], axis=1)

    iota_mat = np.broadcast_to(np.arange(WIN, dtype=np.float32), (128, WIN))
    iota_mat = iota_mat.astype(np_tab)

    ea_perm = np.asarray(edge_attr)[order]  # [ne, 64] in sorted order

    in_maps = []
    meta = dict(w_max=w_max, t_w=t_w, t_pad=t_pad, n_pad0=n_pad0,
                wstart=wstart, w_per_core=w_per_core, np_tab=np_tab)

    for c in range(ncores):
        w0, w1 = int(wstart[c]), int(wstart[c + 1])
        wc = w1 - w0
        e0, e1 = int(cum[w0]), int(cum[w1])
        node_base = w0 * WIN

        # per-edge placement (vectorized)
        j_in_win = np.arange(e0, e1, dtype=np.int64) - cum[win_of_edge[e0:e1]]
        w_loc = (win_of_edge[e0:e1] - w0).astype(np.int64)
        t_glob = w_loc * t_w + (j_in_win >> 7)
        p_slot = (j_in_win & 127)
        flat = p_slot * t_pad + t_glob  # into [128, t_pad]

        off_src = np.zeros((128, t_pad), dtype=np.int32)
        off_dst = np.zeros((128, t_pad), dtype=np.int32)
        dstw = np.full((128, t_pad), -1.0, dtype=np.float32)
        off_src.ravel()[flat] = src_s[e0:e1]
        off_dst.ravel()[flat] = dst_s[e0:e1] - node_base
        dstw.ravel()[flat] = (dst_s[e0:e1] - node_base - w_loc * WIN).astype(np.float32)

        ea_cols = t_glob * 128 + p_slot
        eaT = np.zeros((D + 1, t_pad * 128), dtype=np_tab)
        eaT_body = np.zeros((t_pad * 128, D), dtype=np.float32)
        eaT_body[ea_cols] = ea_perm[e0:e1]
        eaT[:D] = eaT_body.T.astype(np_tab)
        ones_row = np.zeros(t_pad * 128, dtype=np_tab)
        ones_row[ea_cols] = 1.0
        eaT[D] = ones_row

        rng_cols = w_max * WIN
        xT_rng = np.zeros((D + 1, rng_cols), dtype=np_tab)
        hi = min(node_base + rng_cols, n)
        m = hi - node_base
        if m > 0:
            xT_rng[:D, :m] = np.asarray(x)[node_base:hi].T.astype(np_tab)
            xT_rng[D, :m] = 1.0

        in_maps.append({
            "xT_aug": np.ascontiguousarray(xT_aug),
            "xT_rng": np.ascontiguousarray(xT_rng),
            "eaT": np.ascontiguousarray(eaT),
            "off_src": off_src,
            "off_dst": off_dst,
            "dstw": dstw,  # f32: is_equal scalar must be f32
            "iota_mat": np.ascontiguousarray(iota_mat),
            "w_e_pack": w_e_pack,
            "w_qv_pack": w_qv_pack,
            "w_k_pack": w_k_pack,
            "w_s_pack": w_s_pack,
        })

    return in_maps, meta


# ----------------------------------------------------------------------------
# device program
# ----------------------------------------------------------------------------

def build_program(meta, debug=False):
    import concourse.bass as bass
    import concourse.tile as tile
    from concourse import bacc, mybir
    from concourse.bass import IndirectOffsetOnAxis

    w_max = meta["w_max"]
    t_w = meta["t_w"]
    t_pad = meta["t_pad"]
    n_pad0 = meta["n_pad0"]

    dt_tab = mybir.dt.bfloat16 if TABLE_DT == "bf16" else mybir.dt.float32
    f32 = mybir.dt.float32
    i32 = mybir.dt.int32
    AF = mybir.ActivationFunctionType
    ALU = mybir.AluOpType

    rng_cols = w_max * WIN

    nc = bacc.Bacc("TRN2", target_bir_lowering=False, debug=debug)

    d_xT_aug = nc.dram_tensor("xT_aug", [D + 1, n_pad0], dt_tab, kind="ExternalInput")
    d_xT_rng = nc.dram_tensor("xT_rng", [D + 1, rng_cols], dt_tab, kind="ExternalInput")
    d_eaT = nc.dram_tensor("eaT", [D + 1, t_pad * 128], dt_tab, kind="ExternalInput")
    d_off_src = nc.dram_tensor("off_src", [128, t_pad], i32, kind="ExternalInput")
    d_off_dst = nc.dram_tensor("off_dst", [128, t_pad], i32, kind="ExternalInput")
    d_dstw = nc.dram_tensor("dstw", [128, t_pad], f32, kind="ExternalInput")
    d_iota = nc.dram_tensor("iota_mat", [128, WIN], dt_tab, kind="ExternalInput")
    d_w_e = nc.dram_tensor("w_e_pack", [D + 1, D], dt_tab, kind="ExternalInput")
    d_w_qv = nc.dram_tensor("w_qv_pack", [D + 1, 2 * D], dt_tab, kind="ExternalInput")
    d_w_k = nc.dram_tensor("w_k_pack", [D + 1, D], dt_tab, kind="ExternalInput")
    d_w_s = nc.dram_tensor("w_s_pack", [D + 1, D], dt_tab, kind="ExternalInput")
    d_out = nc.dram_tensor("out", [w_max * WIN, D], f32, kind="ExternalOutput")

    dump = os.environ.get("GNN_DUMP", "0") == "1"
    if dump:
        nd = min(16, t_pad)  # tiles dumped
        d_dbg_ktab = nc.dram_tensor("dbg_ktab", [rng_cols, D], f32,
                                    kind="ExternalOutput")
        d_dbg_qvtab = nc.dram_tensor("dbg_qvtab", [min(2048, n_pad0), 2 * D], f32,
                                     kind="ExternalOutput")
        d_dbg_kd = nc.dram_tensor("dbg_kd", [128, nd * D], f32,
                                  kind="ExternalOutput")
        d_dbg_qv = nc.dram_tensor("dbg_qv", [128, nd * 2 * D], f32,
                                  kind="ExternalOutput")
        d_dbg_msg = nc.dram_tensor("dbg_msg", [128, nd * D], f32,
                                   kind="ExternalOutput")
        d_dbg_sel = nc.dram_tensor("dbg_sel", [128, nd * WIN], f32,
                                   kind="ExternalOutput")

    with tile.TileContext(nc) as tc:
        with (
            tc.tile_pool(name="dram", bufs=1, space="DRAM") as dpool,
            tc.tile_pool(name="const", bufs=1) as cpool,
        ):
            qv_tab = dpool.tile([n_pad0, 2 * D], dt_tab)
            k_tab = dpool.tile([rng_cols, D], dt_tab)

            iota_sb = cpool.tile([128, WIN], dt_tab)
            w_e_sb = cpool.tile([D + 1, D], dt_tab)
            w_qv_sb = cpool.tile([D + 1, 2 * D], dt_tab)
            w_k_sb = cpool.tile([D + 1, D], dt_tab)
            w_s_sb = cpool.tile([D + 1, D], dt_tab)
            xT_rng_sb = cpool.tile([D + 1, rng_cols], dt_tab)
            off_src_sb = cpool.tile([128, t_pad], i32)
            off_dst_sb = cpool.tile([128, t_pad], i32)
            dstw_sb = cpool.tile([128, t_pad], f32)

            nc.sync.dma_start(iota_sb[:], d_iota[:])
            nc.sync.dma_start(w_e_sb[:], d_w_e[:])
            nc.sync.dma_start(w_qv_sb[:], d_w_qv[:])
            nc.sync.dma_start(w_k_sb[:], d_w_k[:])
            nc.sync.dma_start(w_s_sb[:], d_w_s[:])
            nc.sync.dma_start(xT_rng_sb[:], d_xT_rng[:])
            nc.sync.dma_start(off_src_sb[:], d_off_src[:])
            nc.sync.dma_start(off_dst_sb[:], d_off_dst[:])
            nc.sync.dma_start(dstw_sb[:], d_dstw[:])

            # ---------------- pass 0: build qv + k tables ----------------
            n_tiles0 = n_pad0 // 128
            xchunk = 16  # node-tiles per xT_aug load
            with (
                tc.tile_pool(name="p0x", bufs=3) as p0x,
                tc.tile_pool(name="p0o", bufs=4) as p0o,
                tc.tile_pool(name="p0ps", bufs=4, space="PSUM") as p0ps,
            ):
                for c0 in range(0, n_tiles0, xchunk):
                    cn = min(xchunk, n_tiles0 - c0)
                    xa = p0x.tile([D + 1, cn * 128], dt_tab, tag="xa")
                    nc.sync.dma_start(
                        xa[:], d_xT_aug[:, c0 * 128:(c0 + cn) * 128])
                    for j in range(cn):
                        ps = p0ps.tile([128, 2 * D], f32, tag="qvps")
                        nc.tensor.matmul(ps[:], xa[:, j * 128:(j + 1) * 128],
                                         w_qv_sb[:], start=True, stop=True)
                        ob = p0o.tile([128, 2 * D], dt_tab, tag="qvo")
                        nc.vector.tensor_copy(ob[:], ps[:])
                        nt = c0 + j
                        nc.sync.dma_start(
                            qv_tab[nt * 128:(nt + 1) * 128, :], ob[:])
                for w in range(w_max):
                    ps = p0ps.tile([128, D], f32, tag="kps")
                    nc.tensor.matmul(ps[:], xT_rng_sb[:, w * 128:(w + 1) * 128],
                                     w_k_sb[:], start=True, stop=True)
                    ob = p0o.tile([128, D], dt_tab, tag="ko")
                    nc.vector.tensor_copy(ob[:], ps[:])
                    nc.sync.dma_start(k_tab[w * 128:(w + 1) * 128, :], ob[:])

            # tables must be fully written before any gather reads them
            # (DRAM RAW deps are not reliably tracked through indirect DMA)
            tc.strict_bb_all_engine_barrier()

            if dump:
                with tc.tile_pool(name="dbg", bufs=2) as dbgp:
                    for w in range(w_max):
                        tb = dbgp.tile([128, D], dt_tab, tag="dk")
                        nc.sync.dma_start(tb[:], k_tab[w * 128:(w + 1) * 128, :])
                        tf = dbgp.tile([128, D], f32, tag="dkf")
                        nc.vector.tensor_copy(tf[:], tb[:])
                        nc.sync.dma_start(d_dbg_ktab[w * 128:(w + 1) * 128, :], tf[:])
                    for w in range(min(2048, n_pad0) // 128):
                        tb = dbgp.tile([128, 2 * D], dt_tab, tag="dq")
                        nc.sync.dma_start(tb[:], qv_tab[w * 128:(w + 1) * 128, :])
                        tf = dbgp.tile([128, 2 * D], f32, tag="dqf")
                        nc.vector.tensor_copy(tf[:], tb[:])
                        nc.sync.dma_start(d_dbg_qvtab[w * 128:(w + 1) * 128, :], tf[:])

            # ---------------- main loop ----------------
            n_groups = _cdiv(t_pad, GROUP_TILES)
            with (
                tc.tile_pool(name="gath", bufs=2) as gpool,
                tc.tile_pool(name="work", bufs=4) as spool,
                tc.tile_pool(name="eps", bufs=4, space="PSUM") as eps_pool,
                tc.tile_pool(name="wps", bufs=4, space="PSUM") as wps_pool,
                tc.tile_pool(name="outp", bufs=4) as opool,
            ):
                win_ps = None
                for g in range(n_groups):
                    g0 = g * GROUP_TILES
                    nt = min(GROUP_TILES, t_pad - g0)
                    kd_sb = gpool.tile([128, nt * D], dt_tab, tag="kd")
                    qv_sb = gpool.tile([128, nt * 2 * D], dt_tab, tag="qv")
                    ea_sb = gpool.tile([D + 1, nt * 128], dt_tab, tag="ea")
                    for j in range(nt):
                        t0 = g0 + j
                        nc.gpsimd.indirect_dma_start(
                            out=kd_sb[:, j * D:(j + 1) * D], out_offset=None,
                            in_=k_tab[:, :],
                            in_offset=IndirectOffsetOnAxis(
                                ap=off_dst_sb[:, t0:t0 + 1], axis=0))
                        nc.gpsimd.indirect_dma_start(
                            out=qv_sb[:, j * 2 * D:(j + 1) * 2 * D],
                            out_offset=None,
                            in_=qv_tab[:, :],
                            in_offset=IndirectOffsetOnAxis(
                                ap=off_src_sb[:, t0:t0 + 1], axis=0))
                    nc.sync.dma_start(
                        ea_sb[:], d_eaT[:, g0 * 128:(g0 + nt) * 128])
                    if dump and g == 0:
                        dkd = gpool.tile([128, nd * D], f32, tag="dkd")
                        nc.vector.tensor_copy(dkd[:], kd_sb[:, :nd * D])
                        nc.sync.dma_start(d_dbg_kd[:], dkd[:])
                        dqv = gpool.tile([128, nd * 2 * D], f32, tag="dqv")
                        nc.vector.tensor_copy(dqv[:], qv_sb[:, :nd * 2 * D])
                        nc.sync.dma_start(d_dbg_qv[:], dqv[:])

                    for j in range(nt):
                        t = g0 + j
                        w = t // t_w
                        first = (t % t_w == 0)
                        last = (t % t_w == t_w - 1)
                        kd = kd_sb[:, j * D:(j + 1) * D]
                        qs = qv_sb[:, j * 2 * D:j * 2 * D + D]
                        vs = qv_sb[:, j * 2 * D + D:(j + 1) * 2 * D]
                        ea_t = ea_sb[:, j * 128:(j + 1) * 128]

                        e_ps = eps_pool.tile([128, D], f32, tag="eps")
                        nc.tensor.matmul(e_ps[:], ea_t, w_e_sb[:],
                                         start=True, stop=True)
                        e_sb = spool.tile([128, D], dt_tab, tag="e")
                        nc.scalar.activation(e_sb[:], e_ps[:], AF.Copy)

                        s_sb = spool.tile([128, D], dt_tab, tag="s")
                        nc.vector.tensor_add(s_sb[:], kd, qs)
                        tt_sb = spool.tile([128, D], dt_tab, tag="t")
                        nc.vector.tensor_add(tt_sb[:], s_sb[:], e_sb[:])
                        g_sb = spool.tile([128, D], dt_tab, tag="g")
                        nc.scalar.activation(g_sb[:], tt_sb[:], AF.Sigmoid,
                                             scale=2.0)
                        vv_sb = spool.tile([128, D], dt_tab, tag="vv")
                        nc.vector.tensor_add(vv_sb[:], vs, e_sb[:])
                        msg_sb = spool.tile([128, D], dt_tab, tag="msg")
                        nc.vector.tensor_mul(msg_sb[:], g_sb[:], vv_sb[:])

                        sel_sb = spool.tile([128, WIN], dt_tab, tag="sel")
                        if SEL_MODE == "vector":
                            sel_eng = nc.vector
                        elif SEL_MODE == "gpsimd":
                            sel_eng = nc.gpsimd
                        else:
                            sel_eng = nc.vector if (t % 2 == 0) else nc.gpsimd
                        sel_eng.tensor_scalar(
                            sel_sb[:], iota_sb[:], dstw_sb[:, t:t + 1], None,
                            ALU.is_equal)

                        if dump and t < nd:
                            dms = spool.tile([128, D], f32, tag="dms")
                            nc.vector.tensor_copy(dms[:], msg_sb[:])
                            nc.sync.dma_start(
                                d_dbg_msg[:, t * D:(t + 1) * D], dms[:])
                            dsl = spool.tile([128, WIN], f32, tag="dsl")
                            nc.vector.tensor_copy(dsl[:], sel_sb[:])
                            nc.sync.dma_start(
                                d_dbg_sel[:, t * WIN:(t + 1) * WIN], dsl[:])
                        if first:
                            win_ps = wps_pool.tile([128, D], f32, tag="win")
                        nc.tensor.matmul(win_ps[:], sel_sb[:], msg_sb[:],
                                         start=first, stop=False)
                        if last:
                            nc.tensor.matmul(
                                win_ps[:],
                                xT_rng_sb[:, w * 128:(w + 1) * 128],
                                w_s_sb[:], start=False, stop=True)
                            out_sb = opool.tile([128, D], f32, tag="out")
                            nc.scalar.activation(out_sb[:], win_ps[:], AF.Relu)
                            nc.sync.dma_start(
                                d_out[w * 128:(w + 1) * 128, :], out_sb[:])

    nc.compile()
    return nc


# ----------------------------------------------------------------------------
# entry point
# ----------------------------------------------------------------------------

def kernel(x, edge_index, edge_attr, u, batch,
           Wk, bk, Wq, bq, Wv, bv, We, be, Ws, bs):
    x = np.asarray(x)
    edge_index_np = np.asarray(edge_index)
    edge_attr = np.asarray(edge_attr)
    n = x.shape[0]

    in_maps, meta = build_host_data(
        x, edge_index_np, edge_attr, Wk, bk, Wq, bq, Wv, bv, We, be, Ws, bs,
        n, NCORES)

    nc = build_program(meta, debug=False)

    from concourse import bass_utils
    trace = os.environ.get("GNN_TRACE", "0") == "1"
    res = bass_utils.run_bass_kernel_spmd(
        nc, in_maps, core_ids=list(range(NCORES)), trace=trace)
    if trace:
        kernel.last_exec_time_ns = res.exec_time_ns
        print(f"[kernel] exec_time_ns = {res.exec_time_ns}")

    outs = res.results
    full = np.empty((n, D), dtype=np.float32)
    wstart = meta["wstart"]
    for c in range(NCORES):
        base = int(wstart[c]) * WIN
        hi = min(int(wstart[c + 1]) * WIN, n)
        if hi > base:
            full[base:hi] = outs[c]["out"][:hi - base]

    return (full,
            np.asarray(edge_attr),
            np.asarray(u),
            np.asarray(edge_index))


kernel.last_exec_time_ns = None


# revision 19
# speedup vs baseline: 1.7133x; 1.7133x over previous
"""ResGatedGraphConv (gnn_message_passing) Trainium2 Bass kernel, 8-core SPMD.

Strategy:
  - Host: sort edges by dst; shard 128-node windows contiguously over 8 cores
    balancing edge counts (each core owns a node range -> no all-reduce).
  - Device per core:
      pass0: build qv table [N,128] = [q/2 | v] and k/2 table for local range
             (PE matmuls from host-pretransposed x).
      loop: per 128-edge tile: gather k/2[dst_local], qv[src] (indirect DMA),
            e = eaT_tile.T @ [We.T;be] (PE, psum), s=(kd+qs) t=s+e vv=vs+e
            (DVE), g=sigmoid(2t) (ACT), msg=g*vv (DVE),
            sel one-hot = (iota == dst_in_window) (DVE/GPSIMD),
            window psum += sel.T @ msg (PE accumulate).
        per window (T_w tiles): psum += xT_win.T @ [Ws.T;bs] (skip),
            out = relu(psum) (ACT), DMA out rows.
  - Host: concat per-core row ranges.
"""

import os
import sys
import math

import numpy as np

for _p in ("/opt/trn_rl_repo",):
    if _p not in sys.path:
        sys.path.append(_p)

import ml_dtypes  # noqa: E402

BF16 = ml_dtypes.bfloat16

# problem constants (hardcoded per spec)
N_NODES = 100000
N_EDGES = 1000000
D = 64
NCORES = 8
WIN = 128  # nodes per aggregation window (= one-hot selector width)

# tunables (env-overridable for experiments)
GROUP_TILES = int(os.environ.get("GNN_GROUP_TILES", "64"))  # tiles per gather group
SEL_MODE = os.environ.get("GNN_SEL", "vector")  # vector | gpsimd | alt
TABLE_DT = os.environ.get("GNN_TABLE_DT", "bf16")  # bf16 | f32  (tables+compute)


def _cdiv(a, b):
    return (a + b - 1) // b


# ----------------------------------------------------------------------------
# host-side preprocessing
# ----------------------------------------------------------------------------

def build_host_data(x, edge_index, edge_attr, Wk, bk, Wq, bq, Wv, bv, We, be,
                    Ws, bs, n_nodes, ncores):
    """Sort/shard edges, build per-core input arrays + structural meta."""
    n = n_nodes
    src = np.asarray(edge_index[0], dtype=np.int64)
    dst = np.asarray(edge_index[1], dtype=np.int64)
    ne = src.shape[0]

    order = np.argsort(dst, kind="stable")
    src_s = src[order].astype(np.int32)
    dst_s = dst[order].astype(np.int32)

    w_total = _cdiv(n, WIN)
    win_of_edge = dst_s >> 7
    counts = np.bincount(win_of_edge, minlength=w_total).astype(np.int64)
    cum = np.concatenate([[0], np.cumsum(counts)])

    # contiguous window runs with ~equal edge counts
    targets = (np.arange(1, ncores) * ne) // ncores
    cuts = np.searchsorted(cum, targets)  # window index cuts
    wstart = np.concatenate([[0], cuts, [w_total]]).astype(np.int64)
    w_per_core = np.diff(wstart)
    w_max = int(w_per_core.max())
    t_w = int(max(1, _cdiv(int(counts.max()), 128)))  # tiles per window (global max)
    t_pad = w_max * t_w

    np_tab = BF16 if TABLE_DT == "bf16" else np.float32

    # global padded xT (for qv table build); [65, n_pad0]
    n_pad0 = w_total * WIN
    xT_aug = np.zeros((D + 1, n_pad0), dtype=np_tab)
    xT_aug[:D, :n] = x.T.astype(np_tab)
    xT_aug[D, :n] = 1.0

    # weight packs
    def pack(w, b, scale):
        p = np.zeros((D + 1, D), dtype=np_tab)
        p[:D] = (w.T * scale).astype(np_tab)
        p[D] = (b * scale).astype(np_tab)
        return p

    w_e_pack = pack(We, be, 1.0)
    w_k_pack = pack(Wk, bk, 0.5)
    w_s_pack = pack(Ws, bs, 1.0)
    w_qv_pack = np.concatenate([pack(Wq, bq, 0.5), pack(Wv, bv, 1.0)], axis=1)

    iota_mat = np.broadcast_to(np.arange(WIN, dtype=np.float32), (128, WIN))
    iota_mat = iota_mat.astype(np_tab)

    ea_perm = np.asarray(edge_attr)[order]  # [ne, 64] in sorted order

    in_maps = []
    meta = dict(w_max=w_max, t_w=t_w, t_pad=t_pad, n_pad0=n_pad0,
                wstart=wstart, w_per_core=w_per_core, np_tab=np_tab)

    for c in range(ncores):
        w0, w1 = int(wstart[c]), int(wstart[c + 1])
        wc = w1 - w0
        e0, e1 = int(cum[w0]), int(cum[w1])
        node_base = w0 * WIN

        # per-edge placement (vectorized)
        j_in_win = np.arange(e0, e1, dtype=np.int64) - cum[win_of_edge[e0:e1]]
        w_loc = (win_of_edge[e0:e1] - w0).astype(np.int64)
        t_glob = w_loc * t_w + (j_in_win >> 7)
        p_slot = (j_in_win & 127)
        flat = p_slot * t_pad + t_glob  # into [128, t_pad]

        off_src = np.zeros((128, t_pad), dtype=np.int32)
        dstw = np.full((128, t_pad), -1.0, dtype=np.float32)
        off_src.ravel()[flat] = src_s[e0:e1]
        dstw.ravel()[flat] = (dst_s[e0:e1] - node_base - w_loc * WIN).astype(np.float32)
        # dstw_flat[0, t*128 + p] = dstw[p, t]  (row vector per tile)
        dstw_flat = np.ascontiguousarray(dstw.T).reshape(1, t_pad * 128)

        ea_cols = t_glob * 128 + p_slot
        eaT = np.zeros((D + 1, t_pad * 128), dtype=np_tab)
        eaT_body = np.zeros((t_pad * 128, D), dtype=np.float32)
        eaT_body[ea_cols] = ea_perm[e0:e1]
        eaT[:D] = eaT_body.T.astype(np_tab)
        ones_row = np.zeros(t_pad * 128, dtype=np_tab)
        ones_row[ea_cols] = 1.0
        eaT[D] = ones_row

        rng_cols = w_max * WIN
        xT_rng = np.zeros((D + 1, rng_cols), dtype=np_tab)
        hi = min(node_base + rng_cols, n)
        m = hi - node_base
        if m > 0:
            xT_rng[:D, :m] = np.asarray(x)[node_base:hi].T.astype(np_tab)
            xT_rng[D, :m] = 1.0

        in_maps.append({
            "xT_aug": np.ascontiguousarray(xT_aug),
            "xT_rng": np.ascontiguousarray(xT_rng),
            "eaT": np.ascontiguousarray(eaT),
            "off_src": off_src,
            "dstw": dstw,  # f32: is_equal scalar must be f32
            "dstw_flat": dstw_flat.astype(np_tab),
            "iota_col": np.arange(128, dtype=np.float32).reshape(128, 1),
            "ones_row": np.ones((3, 128), dtype=np_tab),
            "iota_mat": np.ascontiguousarray(iota_mat),
            "w_e_pack": w_e_pack,
            "w_qv_pack": w_qv_pack,
            "w_k_pack": w_k_pack,
            "w_s_pack": w_s_pack,
        })

    return in_maps, meta


# ----------------------------------------------------------------------------
# device program
# ----------------------------------------------------------------------------

def build_program(meta, debug=False):
    import concourse.bass as bass
    import concourse.tile as tile
    from concourse import bacc, mybir
    from concourse.bass import IndirectOffsetOnAxis

    w_max = meta["w_max"]
    t_w = meta["t_w"]
    t_pad = meta["t_pad"]
    n_pad0 = meta["n_pad0"]

    dt_tab = mybir.dt.bfloat16 if TABLE_DT == "bf16" else mybir.dt.float32
    f32 = mybir.dt.float32
    i32 = mybir.dt.int32
    AF = mybir.ActivationFunctionType
    ALU = mybir.AluOpType

    rng_cols = w_max * WIN

    nc = bacc.Bacc("TRN2", target_bir_lowering=False, debug=debug)

    d_xT_aug = nc.dram_tensor("xT_aug", [D + 1, n_pad0], dt_tab, kind="ExternalInput")
    d_xT_rng = nc.dram_tensor("xT_rng", [D + 1, rng_cols], dt_tab, kind="ExternalInput")
    d_eaT = nc.dram_tensor("eaT", [D + 1, t_pad * 128], dt_tab, kind="ExternalInput")
    d_off_src = nc.dram_tensor("off_src", [128, t_pad], i32, kind="ExternalInput")
    d_dstw = nc.dram_tensor("dstw", [128, t_pad], f32, kind="ExternalInput")
    d_dstw_flat = nc.dram_tensor("dstw_flat", [1, t_pad * 128], dt_tab,
                                 kind="ExternalInput")
    d_iota = nc.dram_tensor("iota_mat", [128, WIN], dt_tab, kind="ExternalInput")
    d_iota_col = nc.dram_tensor("iota_col", [128, 1], f32, kind="ExternalInput")
    d_ones = nc.dram_tensor("ones_row", [3, 128], dt_tab, kind="ExternalInput")
    d_w_e = nc.dram_tensor("w_e_pack", [D + 1, D], dt_tab, kind="ExternalInput")
    d_w_qv = nc.dram_tensor("w_qv_pack", [D + 1, 2 * D], dt_tab, kind="ExternalInput")
    d_w_k = nc.dram_tensor("w_k_pack", [D + 1, D], dt_tab, kind="ExternalInput")
    d_w_s = nc.dram_tensor("w_s_pack", [D + 1, D], dt_tab, kind="ExternalInput")
    d_out = nc.dram_tensor("out", [w_max * WIN, D], f32, kind="ExternalOutput")

    with tile.TileContext(nc) as tc:
        with (
            tc.tile_pool(name="dram", bufs=1, space="DRAM") as dpool,
            tc.tile_pool(name="const", bufs=1) as cpool,
        ):
            qv_tab = dpool.tile([n_pad0, 2 * D], dt_tab)

            iota_sb = cpool.tile([128, WIN], dt_tab)
            iota_col_sb = cpool.tile([128, 1], f32)
            ones_sb = cpool.tile([65, 128], dt_tab)
            w_e_sb = cpool.tile([D + 1, D], dt_tab)
            w_qv_sb = cpool.tile([D + 1, 2 * D], dt_tab)
            w_k_sb = cpool.tile([D + 1, D], dt_tab)
            w_s_sb = cpool.tile([D + 1, D], dt_tab)
            xT_rng_sb = cpool.tile([D + 1, rng_cols], dt_tab)
            off_src_sb = cpool.tile([128, t_pad], i32)
            dstw_sb = cpool.tile([128, t_pad], f32)

            nc.sync.dma_start(iota_sb[:], d_iota[:])
            nc.sync.dma_start(iota_col_sb[:], d_iota_col[:])
            for r in range(3):
                nc.sync.dma_start(ones_sb[32 * r:32 * r + 1, :],
                                  d_ones[r:r + 1, :])
            nc.sync.dma_start(w_e_sb[:], d_w_e[:])
            nc.sync.dma_start(w_qv_sb[:], d_w_qv[:])
            nc.sync.dma_start(w_k_sb[:], d_w_k[:])
            nc.sync.dma_start(w_s_sb[:], d_w_s[:])
            nc.sync.dma_start(xT_rng_sb[:], d_xT_rng[:])
            nc.sync.dma_start(off_src_sb[:], d_off_src[:])
            nc.sync.dma_start(dstw_sb[:], d_dstw[:])

            # ---------------- pass 0: build qv table ----------------
            n_tiles0 = n_pad0 // 128
            xchunk = 16  # node-tiles per xT_aug load
            with (
                tc.tile_pool(name="p0x", bufs=3) as p0x,
                tc.tile_pool(name="p0o", bufs=4) as p0o,
                tc.tile_pool(name="p0ps", bufs=4, space="PSUM") as p0ps,
            ):
                for c0 in range(0, n_tiles0, xchunk):
                    cn = min(xchunk, n_tiles0 - c0)
                    xa = p0x.tile([D + 1, cn * 128], dt_tab, tag="xa")
                    nc.sync.dma_start(
                        xa[:], d_xT_aug[:, c0 * 128:(c0 + cn) * 128])
                    for j in range(cn):
                        ps = p0ps.tile([128, 2 * D], f32, tag="qvps")
                        nc.tensor.matmul(ps[:], xa[:, j * 128:(j + 1) * 128],
                                         w_qv_sb[:], start=True, stop=True)
                        ob = p0o.tile([128, 2 * D], dt_tab, tag="qvo")
                        nc.vector.tensor_copy(ob[:], ps[:])
                        nt = c0 + j
                        nc.sync.dma_start(
                            qv_tab[nt * 128:(nt + 1) * 128, :], ob[:])

            # table must be fully written before any gather reads it
            # (DRAM RAW deps are not reliably tracked through indirect DMA)
            tc.strict_bb_all_engine_barrier()

            # ---------------- main loop ----------------
            n_groups = _cdiv(t_pad, GROUP_TILES)
            with (
                tc.tile_pool(name="gath", bufs=2) as gpool,
                tc.tile_pool(name="work", bufs=4) as spool,
                tc.tile_pool(name="kwin", bufs=3) as kpool,
                tc.tile_pool(name="evps", bufs=2, space="PSUM") as evps_pool,
                tc.tile_pool(name="eps", bufs=2, space="PSUM") as eps_pool,
                tc.tile_pool(name="bps", bufs=2, space="PSUM") as bps_pool,
                tc.tile_pool(name="wps", bufs=2, space="PSUM") as wps_pool,
                tc.tile_pool(name="outp", bufs=4) as opool,
            ):
                win_ps = None
                k_win_sb = {}
                for g in range(n_groups):
                    g0 = g * GROUP_TILES
                    nt = min(GROUP_TILES, t_pad - g0)
                    qv_sb = gpool.tile([128, nt * 2 * D], dt_tab, tag="qv")
                    ea_sb = gpool.tile([D + 1, nt * 128], dt_tab, tag="ea")
                    for j in range(nt):
                        t0 = g0 + j
                        nc.gpsimd.indirect_dma_start(
                            out=qv_sb[:, j * 2 * D:(j + 1) * 2 * D],
                            out_offset=None,
                            in_=qv_tab[:, :],
                            in_offset=IndirectOffsetOnAxis(
                                ap=off_src_sb[:, t0:t0 + 1], axis=0))
                    nc.sync.dma_start(
                        ea_sb[:], d_eaT[:, g0 * 128:(g0 + nt) * 128])
                    dwrow_sb = gpool.tile([1, nt * 128], dt_tab, tag="dwrow")
                    nc.sync.dma_start(
                        dwrow_sb[:], d_dstw_flat[:, g0 * 128:(g0 + nt) * 128])

                    for q0 in range(0, nt, 4):
                        qn = min(4, nt - q0)
                        # two psum quads: ev stays pure e; eg accumulates
                        # e + k/2[dst] via the selT expansion matmul
                        ev_ps = evps_pool.tile([128, qn * D], f32, tag="ev")
                        e_ps = eps_pool.tile([128, qn * D], f32, tag="eps")
                        for j in range(q0, q0 + qn):
                            t = g0 + j
                            w = t // t_w
                            if (t % t_w == 0) and w < w_max:
                                # k window table for new window
                                kps = bps_pool.tile([128, D], f32, tag="bps")
                                nc.tensor.matmul(
                                    kps[:],
                                    xT_rng_sb[:, w * 128:(w + 1) * 128],
                                    w_k_sb[:], start=True, stop=True)
                                kw = kpool.tile([128, D], dt_tab, tag="kw")
                                nc.vector.tensor_copy(kw[:], kps[:])
                                k_win_sb[w] = kw
                            sl = slice((j - q0) * D, (j - q0 + 1) * D)
                            nc.tensor.matmul(ev_ps[:, sl],
                                             ea_sb[:, j * 128:(j + 1) * 128],
                                             w_e_sb[:], start=(j == q0),
                                             stop=(j == q0 + qn - 1))
                            nc.tensor.matmul(e_ps[:, sl],
                                             ea_sb[:, j * 128:(j + 1) * 128],
                                             w_e_sb[:], start=(j == q0),
                                             stop=False)
                        for j in range(q0, q0 + qn):
                            t = g0 + j
                            w = t // t_w
                            # broadcast dstw across partitions: ones^T @ row
                            b_ps = bps_pool.tile([128, 128], f32, tag="bps")
                            nc.tensor.matmul(
                                b_ps[:], ones_sb[0:1, :],
                                dwrow_sb[:, j * 128:(j + 1) * 128],
                                start=True, stop=True)
                            selT = spool.tile([128, 128], dt_tab, tag="selT")
                            nc.vector.tensor_scalar(
                                selT[:], b_ps[:], iota_col_sb[:], None,
                                ALU.is_equal)
                            sl = slice((j - q0) * D, (j - q0 + 1) * D)
                            nc.tensor.matmul(e_ps[:, sl], selT[:],
                                             k_win_sb[t // t_w][:],
                                             start=False,
                                             stop=(j == q0 + qn - 1))
                        # gate input t = (e + k/2[dst]) + q/2[src]
                        qs_view = qv_sb[:, q0 * 2 * D:(q0 + qn) * 2 * D]
                        qs_view = qs_view.rearrange("p (j two d) -> p j two d",
                                                    two=2, d=D)
                        t_sb = spool.tile([128, qn * D], dt_tab, tag="t")
                        t_view = t_sb[:].rearrange("p (j d) -> p j d", d=D)
                        nc.vector.tensor_tensor(
                            t_view, e_ps[:].rearrange("p (j d) -> p j d", d=D),
                            qs_view[:, :, 0, :], op=ALU.add)
                        g_sb = spool.tile([128, qn * D], dt_tab, tag="g")
                        nc.scalar.activation(g_sb[:], t_sb[:], AF.Sigmoid,
                                             scale=2.0)
                        vv_sb = spool.tile([128, qn * D], dt_tab, tag="vv")
                        nc.vector.tensor_tensor(
                            vv_sb[:].rearrange("p (j d) -> p j d", d=D),
                            qs_view[:, :, 1, :],
                            ev_ps[:].rearrange("p (j d) -> p j d", d=D),
                            op=ALU.add)
                        msg_sb = spool.tile([128, qn * D], dt_tab, tag="msg")
                        nc.vector.tensor_mul(msg_sb[:], g_sb[:], vv_sb[:])

                        for j in range(q0, q0 + qn):
                            t = g0 + j
                            w = t // t_w
                            first = (t % t_w == 0)
                            last = (t % t_w == t_w - 1)
                            sel_sb = spool.tile([128, WIN], dt_tab, tag="sel")
                            sel_eng = nc.vector if SEL_MODE == "vector" \
                                else nc.gpsimd
                            sel_eng.tensor_scalar(
                                sel_sb[:], iota_sb[:], dstw_sb[:, t:t + 1],
                                None, ALU.is_equal)
                            if first:
                                win_ps = wps_pool.tile([128, D], f32,
                                                       tag="win")
                            sl = slice((j - q0) * D, (j - q0 + 1) * D)
                            nc.tensor.matmul(win_ps[:], sel_sb[:],
                                             msg_sb[:, sl],
                                             start=first, stop=False)
                            if last:
                                nc.tensor.matmul(
                                    win_ps[:],
                                    xT_rng_sb[:, w * 128:(w + 1) * 128],
                                    w_s_sb[:], start=False, stop=True)
                                out_sb = opool.tile([128, D], f32, tag="out")
                                nc.scalar.activation(out_sb[:], win_ps[:],
                                                     AF.Relu)
                                nc.sync.dma_start(
                                    d_out[w * 128:(w + 1) * 128, :],
                                    out_sb[:])

    nc.compile()
    return nc


# ----------------------------------------------------------------------------
# entry point
# ----------------------------------------------------------------------------

def kernel(x, edge_index, edge_attr, u, batch,
           Wk, bk, Wq, bq, Wv, bv, We, be, Ws, bs):
    x = np.asarray(x)
    edge_index_np = np.asarray(edge_index)
    edge_attr = np.asarray(edge_attr)
    n = x.shape[0]

    in_maps, meta = build_host_data(
        x, edge_index_np, edge_attr, Wk, bk, Wq, bq, Wv, bv, We, be, Ws, bs,
        n, NCORES)

    nc = build_program(meta, debug=False)

    from concourse import bass_utils
    trace = os.environ.get("GNN_TRACE", "0") == "1"
    res = bass_utils.run_bass_kernel_spmd(
        nc, in_maps, core_ids=list(range(NCORES)), trace=trace)
    if trace:
        kernel.last_exec_time_ns = res.exec_time_ns
        print(f"[kernel] exec_time_ns = {res.exec_time_ns}")

    outs = res.results
    full = np.empty((n, D), dtype=np.float32)
    wstart = meta["wstart"]
    for c in range(NCORES):
        base = int(wstart[c]) * WIN
        hi = min(int(wstart[c + 1]) * WIN, n)
        if hi > base:
            full[base:hi] = outs[c]["out"][:hi - base]

    return (full,
            np.asarray(edge_attr),
            np.asarray(u),
            np.asarray(edge_index))


kernel.last_exec_time_ns = None


# revision 20
# speedup vs baseline: 1.7152x; 1.0011x over previous
"""ResGatedGraphConv (gnn_message_passing) Trainium2 Bass kernel, 8-core SPMD.

Strategy:
  - Host: sort edges by dst; shard 128-node windows contiguously over 8 cores
    balancing edge counts (each core owns a node range -> no all-reduce).
  - Device per core:
      pass0: build qv table [N,128] = [q/2 | v] and k/2 table for local range
             (PE matmuls from host-pretransposed x).
      loop: per 128-edge tile: gather k/2[dst_local], qv[src] (indirect DMA),
            e = eaT_tile.T @ [We.T;be] (PE, psum), s=(kd+qs) t=s+e vv=vs+e
            (DVE), g=sigmoid(2t) (ACT), msg=g*vv (DVE),
            sel one-hot = (iota == dst_in_window) (DVE/GPSIMD),
            window psum += sel.T @ msg (PE accumulate).
        per window (T_w tiles): psum += xT_win.T @ [Ws.T;bs] (skip),
            out = relu(psum) (ACT), DMA out rows.
  - Host: concat per-core row ranges.
"""

import os
import sys

import numpy as np

for _p in ("/opt/trn_rl_repo",):
    if _p not in sys.path:
        sys.path.append(_p)

import ml_dtypes  # noqa: E402

BF16 = ml_dtypes.bfloat16

# problem constants (hardcoded per spec)
N_NODES = 100000
N_EDGES = 1000000
D = 64
NCORES = 8
WIN = 128  # nodes per aggregation window (= one-hot selector width)

# tunables (env-overridable for experiments)
GROUP_TILES = int(os.environ.get("GNN_GROUP_TILES", "64"))  # tiles per gather group
SEL_MODE = os.environ.get("GNN_SEL", "vector")  # vector | gpsimd | alt
TABLE_DT = os.environ.get("GNN_TABLE_DT", "bf16")  # bf16 | f32  (tables+compute)


def _cdiv(a, b):
    return (a + b - 1) // b


# ----------------------------------------------------------------------------
# host-side preprocessing
# ----------------------------------------------------------------------------

def build_host_data(x, edge_index, edge_attr, Wk, bk, Wq, bq, Wv, bv, We, be,
                    Ws, bs, n_nodes, ncores):
    """Sort/shard edges, build per-core input arrays + structural meta."""
    n = n_nodes
    src = np.asarray(edge_index[0], dtype=np.int64)
    dst = np.asarray(edge_index[1], dtype=np.int64)
    ne = src.shape[0]

    order = np.argsort(dst, kind="stable")
    src_s = src[order].astype(np.int32)
    dst_s = dst[order].astype(np.int32)

    w_total = _cdiv(n, WIN)
    win_of_edge = dst_s >> 7
    counts = np.bincount(win_of_edge, minlength=w_total).astype(np.int64)
    cum = np.concatenate([[0], np.cumsum(counts)])

    # contiguous window runs with ~equal edge counts
    targets = (np.arange(1, ncores) * ne) // ncores
    cuts = np.searchsorted(cum, targets)  # window index cuts
    wstart = np.concatenate([[0], cuts, [w_total]]).astype(np.int64)
    w_per_core = np.diff(wstart)
    w_max = int(w_per_core.max())
    t_w = int(max(1, _cdiv(int(counts.max()), 128)))  # tiles per window (global max)
    t_pad = w_max * t_w

    np_tab = BF16 if TABLE_DT == "bf16" else np.float32

    # global padded xT (for qv table build); [65, n_pad0]
    n_pad0 = w_total * WIN
    xT_aug = np.zeros((D + 1, n_pad0), dtype=np_tab)
    xT_aug[:D, :n] = x.T.astype(np_tab)
    xT_aug[D, :n] = 1.0

    # weight packs
    def pack(w, b, scale):
        p = np.zeros((D + 1, D), dtype=np_tab)
        p[:D] = (w.T * scale).astype(np_tab)
        p[D] = (b * scale).astype(np_tab)
        return p

    w_e_pack = pack(We, be, 1.0)
    w_k_pack = pack(Wk, bk, 0.5)
    w_s_pack = pack(Ws, bs, 1.0)
    w_qv_pack = np.concatenate([pack(Wq, bq, 0.5), pack(Wv, bv, 1.0)], axis=1)

    iota_mat = np.broadcast_to(np.arange(WIN, dtype=np.float32), (128, WIN))
    iota_mat = iota_mat.astype(np_tab)

    ea_perm = np.asarray(edge_attr)[order]  # [ne, 64] in sorted order

    in_maps = []
    meta = dict(w_max=w_max, t_w=t_w, t_pad=t_pad, n_pad0=n_pad0,
                wstart=wstart, w_per_core=w_per_core, np_tab=np_tab)

    for c in range(ncores):
        w0, w1 = int(wstart[c]), int(wstart[c + 1])
        wc = w1 - w0
        e0, e1 = int(cum[w0]), int(cum[w1])
        node_base = w0 * WIN

        # per-edge placement (vectorized)
        j_in_win = np.arange(e0, e1, dtype=np.int64) - cum[win_of_edge[e0:e1]]
        w_loc = (win_of_edge[e0:e1] - w0).astype(np.int64)
        t_glob = w_loc * t_w + (j_in_win >> 7)
        p_slot = (j_in_win & 127)
        flat = p_slot * t_pad + t_glob  # into [128, t_pad]

        off_src = np.zeros((128, t_pad), dtype=np.int32)
        dstw = np.full((128, t_pad), -1.0, dtype=np.float32)
        off_src.ravel()[flat] = src_s[e0:e1]
        dstw.ravel()[flat] = (dst_s[e0:e1] - node_base - w_loc * WIN).astype(np.float32)
        # dstw_flat[0, t*128 + p] = dstw[p, t]  (row vector per tile)
        dstw_flat = np.ascontiguousarray(dstw.T).reshape(1, t_pad * 128)

        ea_cols = t_glob * 128 + p_slot
        eaT = np.zeros((D + 1, t_pad * 128), dtype=np_tab)
        eaT_body = np.zeros((t_pad * 128, D), dtype=np.float32)
        eaT_body[ea_cols] = ea_perm[e0:e1]
        eaT[:D] = eaT_body.T.astype(np_tab)
        ones_row = np.zeros(t_pad * 128, dtype=np_tab)
        ones_row[ea_cols] = 1.0
        eaT[D] = ones_row

        rng_cols = w_max * WIN
        xT_rng = np.zeros((D + 1, rng_cols), dtype=np_tab)
        hi = min(node_base + rng_cols, n)
        m = hi - node_base
        if m > 0:
            xT_rng[:D, :m] = np.asarray(x)[node_base:hi].T.astype(np_tab)
            xT_rng[D, :m] = 1.0

        in_maps.append({
            "xT_aug": np.ascontiguousarray(xT_aug),
            "xT_rng": np.ascontiguousarray(xT_rng),
            "eaT": np.ascontiguousarray(eaT),
            "off_src": off_src,
            "dstw": dstw,  # f32: is_equal scalar must be f32
            "dstw_flat": dstw_flat.astype(np_tab),
            "iota_col": np.arange(128, dtype=np.float32).reshape(128, 1),
            "ones_row": np.ones((3, 128), dtype=np_tab),
            "iota_mat": np.ascontiguousarray(iota_mat),
            "w_e_pack": w_e_pack,
            "w_qv_pack": w_qv_pack,
            "w_k_pack": w_k_pack,
            "w_s_pack": w_s_pack,
        })

    return in_maps, meta


# ----------------------------------------------------------------------------
# device program
# ----------------------------------------------------------------------------

def build_program(meta, debug=False):
    import concourse.bass as bass
    import concourse.tile as tile
    from concourse import bacc, mybir
    from concourse.bass import IndirectOffsetOnAxis

    w_max = meta["w_max"]
    t_w = meta["t_w"]
    t_pad = meta["t_pad"]
    n_pad0 = meta["n_pad0"]

    dt_tab = mybir.dt.bfloat16 if TABLE_DT == "bf16" else mybir.dt.float32
    f32 = mybir.dt.float32
    i32 = mybir.dt.int32
    AF = mybir.ActivationFunctionType
    ALU = mybir.AluOpType

    rng_cols = w_max * WIN

    nc = bacc.Bacc("TRN2", target_bir_lowering=False, debug=debug)

    d_xT_aug = nc.dram_tensor("xT_aug", [D + 1, n_pad0], dt_tab, kind="ExternalInput")
    d_xT_rng = nc.dram_tensor("xT_rng", [D + 1, rng_cols], dt_tab, kind="ExternalInput")
    d_eaT = nc.dram_tensor("eaT", [D + 1, t_pad * 128], dt_tab, kind="ExternalInput")
    d_off_src = nc.dram_tensor("off_src", [128, t_pad], i32, kind="ExternalInput")
    d_dstw = nc.dram_tensor("dstw", [128, t_pad], f32, kind="ExternalInput")
    d_dstw_flat = nc.dram_tensor("dstw_flat", [1, t_pad * 128], dt_tab,
                                 kind="ExternalInput")
    d_iota = nc.dram_tensor("iota_mat", [128, WIN], dt_tab, kind="ExternalInput")
    d_iota_col = nc.dram_tensor("iota_col", [128, 1], f32, kind="ExternalInput")
    d_ones = nc.dram_tensor("ones_row", [3, 128], dt_tab, kind="ExternalInput")
    d_w_e = nc.dram_tensor("w_e_pack", [D + 1, D], dt_tab, kind="ExternalInput")
    d_w_qv = nc.dram_tensor("w_qv_pack", [D + 1, 2 * D], dt_tab, kind="ExternalInput")
    d_w_k = nc.dram_tensor("w_k_pack", [D + 1, D], dt_tab, kind="ExternalInput")
    d_w_s = nc.dram_tensor("w_s_pack", [D + 1, D], dt_tab, kind="ExternalInput")
    d_out = nc.dram_tensor("out", [w_max * WIN, D], f32, kind="ExternalOutput")

    with tile.TileContext(nc) as tc:
        with (
            tc.tile_pool(name="dram", bufs=1, space="DRAM") as dpool,
            tc.tile_pool(name="const", bufs=1) as cpool,
        ):
            qv_tab = dpool.tile([n_pad0, 2 * D], dt_tab)

            iota_sb = cpool.tile([128, WIN], dt_tab)
            iota_col_sb = cpool.tile([128, 1], f32)
            ones_sb = cpool.tile([65, 128], dt_tab)
            w_e_sb = cpool.tile([D + 1, D], dt_tab)
            w_qv_sb = cpool.tile([D + 1, 2 * D], dt_tab)
            w_k_sb = cpool.tile([D + 1, D], dt_tab)
            w_s_sb = cpool.tile([D + 1, D], dt_tab)
            xT_rng_sb = cpool.tile([D + 1, rng_cols], dt_tab)
            off_src_sb = cpool.tile([128, t_pad], i32)
            dstw_sb = cpool.tile([128, t_pad], f32)

            nc.sync.dma_start(iota_sb[:], d_iota[:])
            nc.sync.dma_start(iota_col_sb[:], d_iota_col[:])
            for r in range(3):
                nc.sync.dma_start(ones_sb[32 * r:32 * r + 1, :],
                                  d_ones[r:r + 1, :])
            nc.sync.dma_start(w_e_sb[:], d_w_e[:])
            nc.sync.dma_start(w_qv_sb[:], d_w_qv[:])
            nc.sync.dma_start(w_k_sb[:], d_w_k[:])
            nc.sync.dma_start(w_s_sb[:], d_w_s[:])
            nc.sync.dma_start(xT_rng_sb[:], d_xT_rng[:])
            nc.sync.dma_start(off_src_sb[:], d_off_src[:])
            nc.sync.dma_start(dstw_sb[:], d_dstw[:])

            # ---------------- pass 0: build qv table ----------------
            n_tiles0 = n_pad0 // 128
            xchunk = 16  # node-tiles per xT_aug load
            with (
                tc.tile_pool(name="p0x", bufs=3) as p0x,
                tc.tile_pool(name="p0o", bufs=4) as p0o,
                tc.tile_pool(name="p0ps", bufs=4, space="PSUM") as p0ps,
            ):
                for c0 in range(0, n_tiles0, xchunk):
                    cn = min(xchunk, n_tiles0 - c0)
                    xa = p0x.tile([D + 1, cn * 128], dt_tab, tag="xa")
                    nc.sync.dma_start(
                        xa[:], d_xT_aug[:, c0 * 128:(c0 + cn) * 128])
                    for j in range(cn):
                        ps = p0ps.tile([128, 2 * D], f32, tag="qvps")
                        nc.tensor.matmul(ps[:], xa[:, j * 128:(j + 1) * 128],
                                         w_qv_sb[:], start=True, stop=True)
                        ob = p0o.tile([128, 2 * D], dt_tab, tag="qvo")
                        nc.vector.tensor_copy(ob[:], ps[:])
                        nt = c0 + j
                        nc.sync.dma_start(
                            qv_tab[nt * 128:(nt + 1) * 128, :], ob[:])

            # table must be fully written before any gather reads it
            # (DRAM RAW deps are not reliably tracked through indirect DMA)
            tc.strict_bb_all_engine_barrier()

            # ---------------- main loop ----------------
            n_groups = _cdiv(t_pad, GROUP_TILES)
            with (
                tc.tile_pool(name="gath", bufs=2) as gpool,
                tc.tile_pool(name="work", bufs=4) as spool,
                tc.tile_pool(name="kwin", bufs=3) as kpool,
                tc.tile_pool(name="evps", bufs=2, space="PSUM") as evps_pool,
                tc.tile_pool(name="eps", bufs=2, space="PSUM") as eps_pool,
                tc.tile_pool(name="bps", bufs=2, space="PSUM") as bps_pool,
                tc.tile_pool(name="wps", bufs=2, space="PSUM") as wps_pool,
                tc.tile_pool(name="outp", bufs=4) as opool,
            ):
                win_ps = None
                k_win_sb = {}
                for g in range(n_groups):
                    g0 = g * GROUP_TILES
                    nt = min(GROUP_TILES, t_pad - g0)
                    qv_sb = gpool.tile([128, nt * 2 * D], dt_tab, tag="qv")
                    ea_sb = gpool.tile([D + 1, nt * 128], dt_tab, tag="ea")
                    for j in range(nt):
                        t0 = g0 + j
                        nc.gpsimd.indirect_dma_start(
                            out=qv_sb[:, j * 2 * D:(j + 1) * 2 * D],
                            out_offset=None,
                            in_=qv_tab[:, :],
                            in_offset=IndirectOffsetOnAxis(
                                ap=off_src_sb[:, t0:t0 + 1], axis=0))
                    nc.sync.dma_start(
                        ea_sb[:], d_eaT[:, g0 * 128:(g0 + nt) * 128])
                    dwrow_sb = gpool.tile([1, nt * 128], dt_tab, tag="dwrow")
                    nc.sync.dma_start(
                        dwrow_sb[:], d_dstw_flat[:, g0 * 128:(g0 + nt) * 128])

                    for q0 in range(0, nt, 4):
                        qn = min(4, nt - q0)
                        # two psum quads: ev stays pure e; eg accumulates
                        # e + k/2[dst] via the selT expansion matmul
                        ev_ps = evps_pool.tile([128, qn * D], f32, tag="ev")
                        e_ps = eps_pool.tile([128, qn * D], f32, tag="eps")
                        for j in range(q0, q0 + qn):
                            t = g0 + j
                            w = t // t_w
                            if (t % t_w == 0) and w < w_max:
                                # k window table for new window
                                kps = bps_pool.tile([128, D], f32, tag="bps")
                                nc.tensor.matmul(
                                    kps[:],
                                    xT_rng_sb[:, w * 128:(w + 1) * 128],
                                    w_k_sb[:], start=True, stop=True)
                                kw = kpool.tile([128, D], dt_tab, tag="kw")
                                nc.vector.tensor_copy(kw[:], kps[:])
                                k_win_sb[w] = kw
                            sl = slice((j - q0) * D, (j - q0 + 1) * D)
                            nc.tensor.matmul(ev_ps[:, sl],
                                             ea_sb[:, j * 128:(j + 1) * 128],
                                             w_e_sb[:], start=(j == q0),
                                             stop=(j == q0 + qn - 1))
                            nc.tensor.matmul(e_ps[:, sl],
                                             ea_sb[:, j * 128:(j + 1) * 128],
                                             w_e_sb[:], start=(j == q0),
                                             stop=False)
                        for j in range(q0, q0 + qn):
                            t = g0 + j
                            w = t // t_w
                            # broadcast dstw across partitions: ones^T @ row
                            b_ps = bps_pool.tile([128, 128], f32, tag="bps")
                            nc.tensor.matmul(
                                b_ps[:], ones_sb[0:1, :],
                                dwrow_sb[:, j * 128:(j + 1) * 128],
                                start=True, stop=True)
                            selT = spool.tile([128, 128], dt_tab, tag="selT")
                            nc.vector.tensor_scalar(
                                selT[:], b_ps[:], iota_col_sb[:], None,
                                ALU.is_equal)
                            sl = slice((j - q0) * D, (j - q0 + 1) * D)
                            nc.tensor.matmul(e_ps[:, sl], selT[:],
                                             k_win_sb[t // t_w][:],
                                             start=False,
                                             stop=(j == q0 + qn - 1))
                        # gate input t = (e + k/2[dst]) + q/2[src]
                        qs_view = qv_sb[:, q0 * 2 * D:(q0 + qn) * 2 * D]
                        qs_view = qs_view.rearrange("p (j two d) -> p j two d",
                                                    two=2, d=D)
                        t_sb = spool.tile([128, qn * D], dt_tab, tag="t")
                        t_view = t_sb[:].rearrange("p (j d) -> p j d", d=D)
                        nc.vector.tensor_tensor(
                            t_view, e_ps[:].rearrange("p (j d) -> p j d", d=D),
                            qs_view[:, :, 0, :], op=ALU.add)
                        g_sb = spool.tile([128, qn * D], dt_tab, tag="g")
                        nc.scalar.activation(g_sb[:], t_sb[:], AF.Sigmoid,
                                             scale=2.0)
                        vv_sb = spool.tile([128, qn * D], dt_tab, tag="vv")
                        nc.vector.tensor_tensor(
                            vv_sb[:].rearrange("p (j d) -> p j d", d=D),
                            qs_view[:, :, 1, :],
                            ev_ps[:].rearrange("p (j d) -> p j d", d=D),
                            op=ALU.add)
                        msg_sb = spool.tile([128, qn * D], dt_tab, tag="msg")
                        nc.vector.tensor_mul(msg_sb[:], g_sb[:], vv_sb[:])

                        for j in range(q0, q0 + qn):
                            t = g0 + j
                            w = t // t_w
                            first = (t % t_w == 0)
                            last = (t % t_w == t_w - 1)
                            sel_sb = spool.tile([128, WIN], dt_tab, tag="sel")
                            sel_eng = nc.vector if SEL_MODE == "vector" \
                                else nc.gpsimd
                            sel_eng.tensor_scalar(
                                sel_sb[:], iota_sb[:], dstw_sb[:, t:t + 1],
                                None, ALU.is_equal)
                            if first:
                                win_ps = wps_pool.tile([128, D], f32,
                                                       tag="win")
                            sl = slice((j - q0) * D, (j - q0 + 1) * D)
                            nc.tensor.matmul(win_ps[:], sel_sb[:],
                                             msg_sb[:, sl],
                                             start=first, stop=False)
                            if last:
                                nc.tensor.matmul(
                                    win_ps[:],
                                    xT_rng_sb[:, w * 128:(w + 1) * 128],
                                    w_s_sb[:], start=False, stop=True)
                                out_sb = opool.tile([128, D], f32, tag="out")
                                nc.scalar.activation(out_sb[:], win_ps[:],
                                                     AF.Relu)
                                nc.sync.dma_start(
                                    d_out[w * 128:(w + 1) * 128, :],
                                    out_sb[:])

    nc.compile()
    return nc


# ----------------------------------------------------------------------------
# entry point
# ----------------------------------------------------------------------------

def kernel(x, edge_index, edge_attr, u, batch,
           Wk, bk, Wq, bq, Wv, bv, We, be, Ws, bs):
    x = np.asarray(x)
    edge_index_np = np.asarray(edge_index)
    edge_attr = np.asarray(edge_attr)
    n = x.shape[0]

    in_maps, meta = build_host_data(
        x, edge_index_np, edge_attr, Wk, bk, Wq, bq, Wv, bv, We, be, Ws, bs,
        n, NCORES)

    nc = build_program(meta, debug=False)

    from concourse import bass_utils
    trace = os.environ.get("GNN_TRACE", "0") == "1"
    res = bass_utils.run_bass_kernel_spmd(
        nc, in_maps, core_ids=list(range(NCORES)), trace=trace)
    if trace:
        kernel.last_exec_time_ns = res.exec_time_ns
        print(f"[kernel] exec_time_ns = {res.exec_time_ns}")

    outs = res.results
    full = np.empty((n, D), dtype=np.float32)
    wstart = meta["wstart"]
    for c in range(NCORES):
        base = int(wstart[c]) * WIN
        hi = min(int(wstart[c + 1]) * WIN, n)
        if hi > base:
            full[base:hi] = outs[c]["out"][:hi - base]

    return (full,
            np.asarray(edge_attr),
            np.asarray(u),
            np.asarray(edge_index))


kernel.last_exec_time_ns = None


# revision 21
# speedup vs baseline: 1.7752x; 1.0350x over previous
"""ResGatedGraphConv (gnn_message_passing) Trainium2 Bass kernel, 8-core SPMD.

Strategy:
  - Host: sort edges by dst; shard 128-node windows contiguously over 8 cores
    balancing edge counts (each core owns a node range -> no all-reduce).
  - Device per core:
      pass0: build qv table [N,128] = [q/2 | v] and k/2 table for local range
             (PE matmuls from host-pretransposed x).
      loop: per 128-edge tile: gather k/2[dst_local], qv[src] (indirect DMA),
            e = eaT_tile.T @ [We.T;be] (PE, psum), s=(kd+qs) t=s+e vv=vs+e
            (DVE), g=sigmoid(2t) (ACT), msg=g*vv (DVE),
            sel one-hot = (iota == dst_in_window) (DVE/GPSIMD),
            window psum += sel.T @ msg (PE accumulate).
        per window (T_w tiles): psum += xT_win.T @ [Ws.T;bs] (skip),
            out = relu(psum) (ACT), DMA out rows.
  - Host: concat per-core row ranges.
"""

import os
import sys

import numpy as np

for _p in ("/opt/trn_rl_repo",):
    if _p not in sys.path:
        sys.path.append(_p)

import ml_dtypes  # noqa: E402

BF16 = ml_dtypes.bfloat16

# problem constants (hardcoded per spec)
N_NODES = 100000
N_EDGES = 1000000
D = 64
NCORES = 8
WIN = 128  # nodes per aggregation window (= one-hot selector width)

# tunables (env-overridable for experiments)
GROUP_TILES = int(os.environ.get("GNN_GROUP_TILES", "48"))  # tiles per gather group
SEL_MODE = os.environ.get("GNN_SEL", "vector")  # vector | gpsimd | alt
TABLE_DT = os.environ.get("GNN_TABLE_DT", "bf16")  # bf16 | f32  (tables+compute)


def _cdiv(a, b):
    return (a + b - 1) // b


# ----------------------------------------------------------------------------
# host-side preprocessing
# ----------------------------------------------------------------------------

def build_host_data(x, edge_index, edge_attr, Wk, bk, Wq, bq, Wv, bv, We, be,
                    Ws, bs, n_nodes, ncores):
    """Sort/shard edges, build per-core input arrays + structural meta."""
    n = n_nodes
    src = np.asarray(edge_index[0], dtype=np.int64)
    dst = np.asarray(edge_index[1], dtype=np.int64)
    ne = src.shape[0]

    order = np.argsort(dst, kind="stable")
    src_s = src[order].astype(np.int32)
    dst_s = dst[order].astype(np.int32)

    w_total = _cdiv(n, WIN)
    win_of_edge = dst_s >> 7
    counts = np.bincount(win_of_edge, minlength=w_total).astype(np.int64)
    cum = np.concatenate([[0], np.cumsum(counts)])

    # contiguous window runs with ~equal edge counts
    targets = (np.arange(1, ncores) * ne) // ncores
    cuts = np.searchsorted(cum, targets)  # window index cuts
    wstart = np.concatenate([[0], cuts, [w_total]]).astype(np.int64)
    w_per_core = np.diff(wstart)
    w_max = int(w_per_core.max())
    t_w = int(max(1, _cdiv(int(counts.max()), 128)))  # tiles per window (global max)
    t_pad = w_max * t_w

    np_tab = BF16 if TABLE_DT == "bf16" else np.float32

    # global padded xT (for qv table build); [65, n_pad0]
    n_pad0 = w_total * WIN
    xT_aug = np.zeros((D + 1, n_pad0), dtype=np_tab)
    xT_aug[:D, :n] = x.T.astype(np_tab)
    xT_aug[D, :n] = 1.0

    # weight packs
    def pack(w, b, scale):
        p = np.zeros((D + 1, D), dtype=np_tab)
        p[:D] = (w.T * scale).astype(np_tab)
        p[D] = (b * scale).astype(np_tab)
        return p

    w_e_pack = pack(We, be, 1.0)
    w_k_pack = pack(Wk, bk, 0.5)
    w_s_pack = pack(Ws, bs, 1.0)
    w_qv_pack = np.concatenate([pack(Wq, bq, 0.5), pack(Wv, bv, 1.0)], axis=1)

    iota_mat = np.broadcast_to(np.arange(WIN, dtype=np.float32), (128, WIN))
    iota_mat = iota_mat.astype(np_tab)

    ea_perm = np.asarray(edge_attr)[order]  # [ne, 64] in sorted order

    in_maps = []
    meta = dict(w_max=w_max, t_w=t_w, t_pad=t_pad, n_pad0=n_pad0,
                wstart=wstart, w_per_core=w_per_core, np_tab=np_tab)

    for c in range(ncores):
        w0, w1 = int(wstart[c]), int(wstart[c + 1])
        wc = w1 - w0
        e0, e1 = int(cum[w0]), int(cum[w1])
        node_base = w0 * WIN

        # per-edge placement (vectorized)
        j_in_win = np.arange(e0, e1, dtype=np.int64) - cum[win_of_edge[e0:e1]]
        w_loc = (win_of_edge[e0:e1] - w0).astype(np.int64)
        t_glob = w_loc * t_w + (j_in_win >> 7)
        p_slot = (j_in_win & 127)
        flat = p_slot * t_pad + t_glob  # into [128, t_pad]

        off_src = np.zeros((128, t_pad), dtype=np.int32)
        dstw = np.full((128, t_pad), -1.0, dtype=np.float32)
        off_src.ravel()[flat] = src_s[e0:e1]
        dstw.ravel()[flat] = (dst_s[e0:e1] - node_base - w_loc * WIN).astype(np.float32)
        # dstw_flat[0, t*128 + p] = dstw[p, t]  (row vector per tile)
        dstw_flat = np.ascontiguousarray(dstw.T).reshape(1, t_pad * 128)

        ea_cols = t_glob * 128 + p_slot
        eaT = np.zeros((D + 1, t_pad * 128), dtype=np_tab)
        eaT_body = np.zeros((t_pad * 128, D), dtype=np.float32)
        eaT_body[ea_cols] = ea_perm[e0:e1]
        eaT[:D] = eaT_body.T.astype(np_tab)
        ones_row = np.zeros(t_pad * 128, dtype=np_tab)
        ones_row[ea_cols] = 1.0
        eaT[D] = ones_row

        rng_cols = w_max * WIN
        xT_rng = np.zeros((D + 1, rng_cols), dtype=np_tab)
        hi = min(node_base + rng_cols, n)
        m = hi - node_base
        if m > 0:
            xT_rng[:D, :m] = np.asarray(x)[node_base:hi].T.astype(np_tab)
            xT_rng[D, :m] = 1.0

        in_maps.append({
            "xT_aug": np.ascontiguousarray(xT_aug),
            "xT_rng": np.ascontiguousarray(xT_rng),
            "eaT": np.ascontiguousarray(eaT),
            "off_src": off_src,
            "dstw": dstw,  # f32: is_equal scalar must be f32
            "dstw_flat": dstw_flat.astype(np_tab),
            "iota_col": np.arange(128, dtype=np.float32).reshape(128, 1),
            "ones_row": np.ones((3, 128), dtype=np_tab),
            "iota_mat": np.ascontiguousarray(iota_mat),
            "w_e_pack": w_e_pack,
            "w_qv_pack": w_qv_pack,
            "w_k_pack": w_k_pack,
            "w_s_pack": w_s_pack,
        })

    return in_maps, meta


# ----------------------------------------------------------------------------
# device program
# ----------------------------------------------------------------------------

def build_program(meta, debug=False):
    import concourse.bass as bass
    import concourse.tile as tile
    from concourse import bacc, mybir
    from concourse.bass import IndirectOffsetOnAxis

    w_max = meta["w_max"]
    t_w = meta["t_w"]
    t_pad = meta["t_pad"]
    n_pad0 = meta["n_pad0"]

    dt_tab = mybir.dt.bfloat16 if TABLE_DT == "bf16" else mybir.dt.float32
    f32 = mybir.dt.float32
    i32 = mybir.dt.int32
    AF = mybir.ActivationFunctionType
    ALU = mybir.AluOpType

    rng_cols = w_max * WIN

    nc = bacc.Bacc("TRN2", target_bir_lowering=False, debug=debug)

    d_xT_aug = nc.dram_tensor("xT_aug", [D + 1, n_pad0], dt_tab, kind="ExternalInput")
    d_xT_rng = nc.dram_tensor("xT_rng", [D + 1, rng_cols], dt_tab, kind="ExternalInput")
    d_eaT = nc.dram_tensor("eaT", [D + 1, t_pad * 128], dt_tab, kind="ExternalInput")
    d_off_src = nc.dram_tensor("off_src", [128, t_pad], i32, kind="ExternalInput")
    d_dstw = nc.dram_tensor("dstw", [128, t_pad], f32, kind="ExternalInput")
    d_dstw_flat = nc.dram_tensor("dstw_flat", [1, t_pad * 128], dt_tab,
                                 kind="ExternalInput")
    d_iota = nc.dram_tensor("iota_mat", [128, WIN], dt_tab, kind="ExternalInput")
    d_iota_col = nc.dram_tensor("iota_col", [128, 1], f32, kind="ExternalInput")
    d_ones = nc.dram_tensor("ones_row", [3, 128], dt_tab, kind="ExternalInput")
    d_w_e = nc.dram_tensor("w_e_pack", [D + 1, D], dt_tab, kind="ExternalInput")
    d_w_qv = nc.dram_tensor("w_qv_pack", [D + 1, 2 * D], dt_tab, kind="ExternalInput")
    d_w_k = nc.dram_tensor("w_k_pack", [D + 1, D], dt_tab, kind="ExternalInput")
    d_w_s = nc.dram_tensor("w_s_pack", [D + 1, D], dt_tab, kind="ExternalInput")
    d_out = nc.dram_tensor("out", [w_max * WIN, D], f32, kind="ExternalOutput")

    with tile.TileContext(nc) as tc:
        with (
            tc.tile_pool(name="dram", bufs=1, space="DRAM") as dpool,
            tc.tile_pool(name="const", bufs=1) as cpool,
        ):
            qv_tab = dpool.tile([n_pad0, 2 * D], dt_tab)

            iota_sb = cpool.tile([128, WIN], dt_tab)
            iota_col_sb = cpool.tile([128, 1], f32)
            ones_sb = cpool.tile([65, 128], dt_tab)
            w_e_sb = cpool.tile([D + 1, D], dt_tab)
            w_qv_sb = cpool.tile([D + 1, 2 * D], dt_tab)
            w_k_sb = cpool.tile([D + 1, D], dt_tab)
            w_s_sb = cpool.tile([D + 1, D], dt_tab)
            xT_rng_sb = cpool.tile([D + 1, rng_cols], dt_tab)
            off_src_sb = cpool.tile([128, t_pad], i32)
            dstw_sb = cpool.tile([128, t_pad], f32)

            nc.sync.dma_start(iota_sb[:], d_iota[:])
            nc.sync.dma_start(iota_col_sb[:], d_iota_col[:])
            for r in range(3):
                nc.sync.dma_start(ones_sb[32 * r:32 * r + 1, :],
                                  d_ones[r:r + 1, :])
            nc.sync.dma_start(w_e_sb[:], d_w_e[:])
            nc.sync.dma_start(w_qv_sb[:], d_w_qv[:])
            nc.sync.dma_start(w_k_sb[:], d_w_k[:])
            nc.sync.dma_start(w_s_sb[:], d_w_s[:])
            nc.sync.dma_start(xT_rng_sb[:], d_xT_rng[:])
            nc.sync.dma_start(off_src_sb[:], d_off_src[:])
            nc.sync.dma_start(dstw_sb[:], d_dstw[:])

            # ---------------- pass 0: build qv table ----------------
            n_tiles0 = n_pad0 // 128
            xchunk = 16  # node-tiles per xT_aug load
            with (
                tc.tile_pool(name="p0x", bufs=3) as p0x,
                tc.tile_pool(name="p0o", bufs=4) as p0o,
                tc.tile_pool(name="p0ps", bufs=4, space="PSUM") as p0ps,
            ):
                for c0 in range(0, n_tiles0, xchunk):
                    cn = min(xchunk, n_tiles0 - c0)
                    xa = p0x.tile([D + 1, cn * 128], dt_tab, tag="xa")
                    nc.sync.dma_start(
                        xa[:], d_xT_aug[:, c0 * 128:(c0 + cn) * 128])
                    for j in range(cn):
                        ps = p0ps.tile([128, 2 * D], f32, tag="qvps")
                        nc.tensor.matmul(ps[:], xa[:, j * 128:(j + 1) * 128],
                                         w_qv_sb[:], start=True, stop=True)
                        ob = p0o.tile([128, 2 * D], dt_tab, tag="qvo")
                        nc.scalar.activation(ob[:], ps[:], AF.Copy)
                        nt = c0 + j
                        # split table writes across both HWDGE rings
                        eng = nc.sync if (nt % 2 == 0) else nc.scalar
                        eng.dma_start(
                            qv_tab[nt * 128:(nt + 1) * 128, :], ob[:])

            # table must be fully written before any gather reads it
            # (DRAM RAW deps are not reliably tracked through indirect DMA)
            tc.strict_bb_all_engine_barrier()

            # ---------------- main loop ----------------
            n_groups = _cdiv(t_pad, GROUP_TILES)
            with (
                tc.tile_pool(name="gath", bufs=3) as gpool,
                tc.tile_pool(name="work", bufs=4) as spool,
                tc.tile_pool(name="kwin", bufs=3) as kpool,
                tc.tile_pool(name="evps", bufs=2, space="PSUM") as evps_pool,
                tc.tile_pool(name="eps", bufs=2, space="PSUM") as eps_pool,
                tc.tile_pool(name="bps", bufs=2, space="PSUM") as bps_pool,
                tc.tile_pool(name="wps", bufs=2, space="PSUM") as wps_pool,
                tc.tile_pool(name="outp", bufs=4) as opool,
            ):
                win_ps = None
                k_win_sb = {}
                for g in range(n_groups):
                    g0 = g * GROUP_TILES
                    nt = min(GROUP_TILES, t_pad - g0)
                    qv_sb = gpool.tile([128, nt * 2 * D], dt_tab, tag="qv")
                    ea_sb = gpool.tile([D + 1, nt * 128], dt_tab, tag="ea")
                    for j in range(nt):
                        t0 = g0 + j
                        nc.gpsimd.indirect_dma_start(
                            out=qv_sb[:, j * 2 * D:(j + 1) * 2 * D],
                            out_offset=None,
                            in_=qv_tab[:, :],
                            in_offset=IndirectOffsetOnAxis(
                                ap=off_src_sb[:, t0:t0 + 1], axis=0))
                    nc.scalar.dma_start(
                        ea_sb[:], d_eaT[:, g0 * 128:(g0 + nt) * 128])
                    dwrow_sb = gpool.tile([1, nt * 128], dt_tab, tag="dwrow")
                    nc.scalar.dma_start(
                        dwrow_sb[:], d_dstw_flat[:, g0 * 128:(g0 + nt) * 128])

                    for q0 in range(0, nt, 4):
                        qn = min(4, nt - q0)
                        # two psum quads: ev stays pure e; eg accumulates
                        # e + k/2[dst] via the selT expansion matmul
                        ev_ps = evps_pool.tile([128, qn * D], f32, tag="ev")
                        e_ps = eps_pool.tile([128, qn * D], f32, tag="eps")
                        for j in range(q0, q0 + qn):
                            t = g0 + j
                            w = t // t_w
                            if (t % t_w == 0) and w < w_max:
                                # k window table for new window
                                kps = bps_pool.tile([128, D], f32, tag="bps")
                                nc.tensor.matmul(
                                    kps[:],
                                    xT_rng_sb[:, w * 128:(w + 1) * 128],
                                    w_k_sb[:], start=True, stop=True)
                                kw = kpool.tile([128, D], dt_tab, tag="kw")
                                nc.vector.tensor_copy(kw[:], kps[:])
                                k_win_sb[w] = kw
                            sl = slice((j - q0) * D, (j - q0 + 1) * D)
                            nc.tensor.matmul(ev_ps[:, sl],
                                             ea_sb[:, j * 128:(j + 1) * 128],
                                             w_e_sb[:], start=(j == q0),
                                             stop=(j == q0 + qn - 1))
                            nc.tensor.matmul(e_ps[:, sl],
                                             ea_sb[:, j * 128:(j + 1) * 128],
                                             w_e_sb[:], start=(j == q0),
                                             stop=False)
                        for j in range(q0, q0 + qn):
                            t = g0 + j
                            w = t // t_w
                            # broadcast dstw across partitions: ones^T @ row
                            b_ps = bps_pool.tile([128, 128], f32, tag="bps")
                            nc.tensor.matmul(
                                b_ps[:], ones_sb[0:1, :],
                                dwrow_sb[:, j * 128:(j + 1) * 128],
                                start=True, stop=True)
                            selT = spool.tile([128, 128], dt_tab, tag="selT")
                            nc.vector.tensor_scalar(
                                selT[:], b_ps[:], iota_col_sb[:], None,
                                ALU.is_equal)
                            sl = slice((j - q0) * D, (j - q0 + 1) * D)
                            nc.tensor.matmul(e_ps[:, sl], selT[:],
                                             k_win_sb[t // t_w][:],
                                             start=False,
                                             stop=(j == q0 + qn - 1))
                        # gate input t = (e + k/2[dst]) + q/2[src]
                        qs_view = qv_sb[:, q0 * 2 * D:(q0 + qn) * 2 * D]
                        qs_view = qs_view.rearrange("p (j two d) -> p j two d",
                                                    two=2, d=D)
                        t_sb = spool.tile([128, qn * D], dt_tab, tag="t")
                        t_view = t_sb[:].rearrange("p (j d) -> p j d", d=D)
                        nc.vector.tensor_tensor(
                            t_view, e_ps[:].rearrange("p (j d) -> p j d", d=D),
                            qs_view[:, :, 0, :], op=ALU.add)
                        g_sb = spool.tile([128, qn * D], dt_tab, tag="g")
                        nc.scalar.activation(g_sb[:], t_sb[:], AF.Sigmoid,
                                             scale=2.0)
                        vv_sb = spool.tile([128, qn * D], dt_tab, tag="vv")
                        nc.vector.tensor_tensor(
                            vv_sb[:].rearrange("p (j d) -> p j d", d=D),
                            qs_view[:, :, 1, :],
                            ev_ps[:].rearrange("p (j d) -> p j d", d=D),
                            op=ALU.add)
                        msg_sb = spool.tile([128, qn * D], dt_tab, tag="msg")
                        nc.vector.tensor_mul(msg_sb[:], g_sb[:], vv_sb[:])

                        for j in range(q0, q0 + qn):
                            t = g0 + j
                            w = t // t_w
                            first = (t % t_w == 0)
                            last = (t % t_w == t_w - 1)
                            sel_sb = spool.tile([128, WIN], dt_tab, tag="sel")
                            sel_eng = nc.vector if SEL_MODE == "vector" \
                                else nc.gpsimd
                            sel_eng.tensor_scalar(
                                sel_sb[:], iota_sb[:], dstw_sb[:, t:t + 1],
                                None, ALU.is_equal)
                            if first:
                                win_ps = wps_pool.tile([128, D], f32,
                                                       tag="win")
                            sl = slice((j - q0) * D, (j - q0 + 1) * D)
                            nc.tensor.matmul(win_ps[:], sel_sb[:],
                                             msg_sb[:, sl],
                                             start=first, stop=False)
                            if last:
                                nc.tensor.matmul(
                                    win_ps[:],
                                    xT_rng_sb[:, w * 128:(w + 1) * 128],
                                    w_s_sb[:], start=False, stop=True)
                                out_sb = opool.tile([128, D], f32, tag="out")
                                nc.scalar.activation(out_sb[:], win_ps[:],
                                                     AF.Relu)
                                nc.sync.dma_start(
                                    d_out[w * 128:(w + 1) * 128, :],
                                    out_sb[:])

    nc.compile()
    return nc


# ----------------------------------------------------------------------------
# entry point
# ----------------------------------------------------------------------------

def kernel(x, edge_index, edge_attr, u, batch,
           Wk, bk, Wq, bq, Wv, bv, We, be, Ws, bs):
    x = np.asarray(x)
    edge_index_np = np.asarray(edge_index)
    edge_attr = np.asarray(edge_attr)
    n = x.shape[0]

    in_maps, meta = build_host_data(
        x, edge_index_np, edge_attr, Wk, bk, Wq, bq, Wv, bv, We, be, Ws, bs,
        n, NCORES)

    nc = build_program(meta, debug=False)

    from concourse import bass_utils
    trace = os.environ.get("GNN_TRACE", "0") == "1"
    res = bass_utils.run_bass_kernel_spmd(
        nc, in_maps, core_ids=list(range(NCORES)), trace=trace)
    if trace:
        kernel.last_exec_time_ns = res.exec_time_ns
        print(f"[kernel] exec_time_ns = {res.exec_time_ns}")

    outs = res.results
    full = np.empty((n, D), dtype=np.float32)
    wstart = meta["wstart"]
    for c in range(NCORES):
        base = int(wstart[c]) * WIN
        hi = min(int(wstart[c + 1]) * WIN, n)
        if hi > base:
            full[base:hi] = outs[c]["out"][:hi - base]

    return (full,
            np.asarray(edge_attr),
            np.asarray(u),
            np.asarray(edge_index))


kernel.last_exec_time_ns = None
